# revision 21
# baseline (speedup 1.0000x reference)
"""Multi-head attention (B=4, N=2048, E=512, H=8) on 8 Trainium2 cores.

Sharding: core c -> (batch b = c//2, head-group g = c%2 of 4 heads).
Each core computes q/k/v projections for its 4 heads, full attention,
and a partial output projection (its heads' slice of Wo contraction);
the host sums the two partials per batch.

Schedule (v2): the kernel is ACT(exp)-floor bound: 4 heads x 16 kc x 2
exps of [128,1024] ~ 128us.  Everything else is arranged to keep ACT
saturated from ~8us and the PE dense (HAM stays at 2.4 GHz):

  - input DMAs split: sync ring [xk cols 0:512, xq, xk 512:1024, xv,
    xk rest], weights on gpsimd ring -> projections start ASAP
  - dummy exp at t=0 preloads the ACT exp table (2.7us off crit path)
  - per head-slot: energy (row-tiled concurrent halves) + exps, with
    the PREVIOUS head's att@v interleaved per-kc (kc-outer accumulation
    into 16 packed PSUM accumulators [128,65] @ stride 80 f32, 6/bank,
    exploiting byte-lazy pending-zero so chains interleave freely)
  - head 0 energy runs both q-halves serially on PE rows 0:63 (no dup
    dependency); heads 1-3 use the dup-swapped copies for pairwise-
    concurrent row-tiled matmuls
  - att tiles are consumed one kc behind production -> ~20 live tiles
  - tail: PE-transpose oall -> ot, Wo partial matmul, staged out-DMA
    on the gpsimd ring
"""

import sys

if "/opt/trn_rl_repo" not in sys.path:
    sys.path.insert(0, "/opt/trn_rl_repo")

import numpy as np

B, N, E, H, D = 4, 2048, 512, 8, 64
NH = 4                      # heads per core
NCHUNK = N // 128           # 16
ECHUNK = E // 128           # 4
SCALE = float(1.0 / np.sqrt(E))
N_CORES = 8

BIG_BUFS = 32               # shared [128,2048] fp16 slots: 12 xT + ~20 att window
ACC_STRIDE = 80             # f32 stride of packed attv accumulators (6 per bank)

_built = None


def _build():
    global _built
    if _built is not None:
        return _built

    from contextlib import ExitStack

    import concourse.bass as bass  # noqa: F401
    import concourse.mybir as mybir
    import concourse.tile as tile
    from concourse import bacc

    DT = mybir.dt.float16
    F32 = mybir.dt.float32
    AF = mybir.ActivationFunctionType

    nc = bacc.Bacc(
        "TRN2",
        target_bir_lowering=False,
        debug=False,
        num_devices=N_CORES,
    )

    xqT = nc.dram_tensor("xqT", [E, N], DT, kind="ExternalInput").ap()
    xkT = nc.dram_tensor("xkT", [E, N], DT, kind="ExternalInput").ap()
    xvT = nc.dram_tensor("xvT", [E, N], DT, kind="ExternalInput").ap()
    wqtd = nc.dram_tensor("wqtd", [E, 256], DT, kind="ExternalInput").ap()
    wktd = nc.dram_tensor("wktd", [E, 256], DT, kind="ExternalInput").ap()
    wvt = nc.dram_tensor("wvt", [E, NH * D], DT, kind="ExternalInput").ap()
    wot = nc.dram_tensor("wot", [NH * D, E], DT, kind="ExternalInput").ap()
    iden = nc.dram_tensor("iden", [128, 128], DT, kind="ExternalInput").ap()
    out = nc.dram_tensor("out", [N, E], F32, kind="ExternalOutput").ap()

    with tile.TileContext(nc) as tc, ExitStack() as ctx:
        consts = ctx.enter_context(tc.tile_pool(name="consts", bufs=1))
        big = ctx.enter_context(tc.tile_pool(name="big", bufs=BIG_BUFS))
        qk = ctx.enter_context(tc.tile_pool(name="qk", bufs=1))
        vp = ctx.enter_context(tc.tile_pool(name="vp", bufs=1))
        oallp = ctx.enter_context(tc.tile_pool(name="oall", bufs=1))
        otp = ctx.enter_context(tc.tile_pool(name="ot", bufs=1))
        ostage = ctx.enter_context(tc.tile_pool(name="ostage", bufs=3))
        smallp = ctx.enter_context(tc.tile_pool(name="small", bufs=4))

        # PSUM (8 banks): energy 2x[128,1024]f32 (4), attv accums 3x[128,512]
        # (3, 16 packed chains), v-proj/warm spare (1).  The tail reuses the
        # energy pool's banks once exps are done.
        ps_big = ctx.enter_context(tc.tile_pool(name="ps_big", bufs=2, space="PSUM"))
        ps_acc = ctx.enter_context(tc.tile_pool(name="ps_acc", bufs=1, space="PSUM"))
        ps_fin = ctx.enter_context(tc.tile_pool(name="ps_fin", bufs=1, space="PSUM"))

        # ---- ACT exp-table preload (dummy) ----
        dume = smallp.tile([128, 8], DT, tag="dume", name="dume")
        dumo = smallp.tile([128, 8], DT, tag="dumo", name="dumo")
        nc.vector.memset(dume[:], 0.0)
        nc.scalar.activation(dumo[:], dume[:], AF.Exp)

        # ---- constant / weight loads (gpsimd ring) ----
        iden_sb = consts.tile([128, 128], DT, tag="iden", name="iden_sb")
        nc.gpsimd.dma_start(out=iden_sb[:], in_=iden[:])
        wq_sb = [consts.tile([128, 256], DT, tag=f"wq{kc}", name=f"wq_sb{kc}") for kc in range(ECHUNK)]
        wk_sb = [consts.tile([128, 256], DT, tag=f"wk{kc}", name=f"wk_sb{kc}") for kc in range(ECHUNK)]
        wv_sb = [consts.tile([128, NH * D], DT, tag=f"wv{kc}", name=f"wv_sb{kc}") for kc in range(ECHUNK)]
        wo_sb = [consts.tile([128, E], DT, tag=f"wo{c}", name=f"wo_sb{c}") for c in range(2)]
        for kc in range(ECHUNK):
            nc.gpsimd.dma_start(out=wq_sb[kc][:], in_=wqtd[128 * kc:128 * (kc + 1), :])
            nc.gpsimd.dma_start(out=wk_sb[kc][:], in_=wktd[128 * kc:128 * (kc + 1), :])
            nc.gpsimd.dma_start(out=wv_sb[kc][:], in_=wvt[128 * kc:128 * (kc + 1), :])
        for c in range(2):
            nc.gpsimd.dma_start(out=wo_sb[c][:], in_=wot[128 * c:128 * (c + 1), :])

        # ---- activation inputs, sync ring (FIFO): xk block0 -> xq -> xk
        # block1 -> xv -> xk blocks 2,3.  xk arrives in 512-col blocks so
        # k-projection of early key chunks can start at ~1.5us.
        xq_sb = [big.tile([128, N], DT, tag="big", name="xq_sb") for _ in range(ECHUNK)]
        xk_sb = [big.tile([128, N], DT, tag="big", name="xk_sb") for _ in range(ECHUNK)]
        xv_sb = [big.tile([128, N], DT, tag="big", name="xv_sb") for _ in range(ECHUNK)]

        def load_xk_block(ns):
            for kc in range(ECHUNK):
                nc.sync.dma_start(
                    out=xk_sb[kc][:, 512 * ns:512 * (ns + 1)],
                    in_=xkT[128 * kc:128 * (kc + 1), 512 * ns:512 * (ns + 1)],
                )

        # critical path: q-proj(head0, cols 0:1024) gates the first exp, so
        # xq half 0 goes first, then the first key block, then the rest.
        for kc in range(ECHUNK):
            nc.sync.dma_start(out=xq_sb[kc][:, 0:1024], in_=xqT[128 * kc:128 * (kc + 1), 0:1024])
        load_xk_block(0)
        for kc in range(ECHUNK):
            nc.sync.dma_start(out=xq_sb[kc][:, 1024:2048], in_=xqT[128 * kc:128 * (kc + 1), 1024:2048])
        load_xk_block(1)
        load_xk_block(2)
        load_xk_block(3)
        for kc in range(ECHUNK):
            nc.sync.dma_start(out=xv_sb[kc][:], in_=xvT[128 * kc:128 * (kc + 1), :])

        # ---- q/k projections ----
        # qnd[mc]/knd[mc] [128, N]: natural head-pair chunks (head 2mc at
        # rows 0:64, head 2mc+1 at 64:128). qdp/kdp are the swapped copies
        # (sbuf->sbuf DMAs) giving each head its 64 dims in BOTH partition
        # halves -> row-tiled energy matmuls run pairwise-concurrent.
        qnd = [qk.tile([128, N], DT, tag=f"qnd{mc}", name="qnd") for mc in range(2)]
        knd = [qk.tile([128, N], DT, tag=f"knd{mc}", name="knd") for mc in range(2)]
        qdp = [qk.tile([128, N], DT, tag=f"qdp{mc}", name="qdp") for mc in range(2)]
        kdp = [qk.tile([128, N], DT, tag=f"kdp{mc}", name="kdp") for mc in range(2)]

        def emit_kproj_block(ns, mc):
            """k-projection for key cols 512ns:512(ns+1) of head pair mc,
            plus the dup-swap of that block."""
            ps = ps_big.tile([128, 1024], F32, tag="big", name="pk")
            for kc in range(ECHUNK):
                nc.tensor.matmul(
                    ps[:, 0:512],
                    wk_sb[kc][:, 128 * mc:128 * (mc + 1)],
                    xk_sb[kc][:, 512 * ns:512 * (ns + 1)],
                    start=(kc == 0),
                    stop=(kc == ECHUNK - 1),
                )
            nc.vector.tensor_copy(knd[mc][:, 512 * ns:512 * (ns + 1)], ps[:, 0:512])
            nc.gpsimd.dma_start(
                out=kdp[mc][0:64, 512 * ns:512 * (ns + 1)],
                in_=knd[mc][64:128, 512 * ns:512 * (ns + 1)],
            )
            nc.gpsimd.dma_start(
                out=kdp[mc][64:128, 512 * ns:512 * (ns + 1)],
                in_=knd[mc][0:64, 512 * ns:512 * (ns + 1)],
            )

        def emit_qproj_half(mc, qh):
            ps = ps_big.tile([128, 1024], F32, tag="big", name="pq")
            for j in range(2):
                for kc in range(ECHUNK):
                    nc.tensor.matmul(
                        ps[:, 512 * j:512 * (j + 1)],
                        wq_sb[kc][:, 128 * mc:128 * (mc + 1)],
                        xq_sb[kc][:, 1024 * qh + 512 * j:1024 * qh + 512 * (j + 1)],
                        start=(kc == 0),
                        stop=(kc == ECHUNK - 1),
                    )
            nc.vector.tensor_copy(qnd[mc][:, 1024 * qh:1024 * (qh + 1)], ps[:])

        def emit_qswap(mc):
            nc.gpsimd.dma_start(out=qdp[mc][0:64, :], in_=qnd[mc][64:128, :])
            nc.gpsimd.dma_start(out=qdp[mc][64:128, :], in_=qnd[mc][0:64, :])

        def half_ap(nd, dp, i, half):
            """[64, N] view of head i's projected data at partition `half`."""
            mc, r = divmod(i, 2)
            if half == 0:
                t = nd[mc] if r == 0 else dp[mc]
                return t[0:64, :]
            t = dp[mc] if r == 0 else nd[mc]
            return t[64:128, :]

        # ---- v projection into augmented layout vsb[kc] [128, NH*65] ----
        # col 65*i + 64 is the ones column for head i (softmax denominator).
        vsb = [vp.tile([128, NH * 65], DT, tag=f"v{mk}", name="v_sb") for mk in range(NCHUNK)]

        def emit_vproj_chunk(mk):
            ps = ps_fin.tile([128, 512], F32, tag="fin", name="psf")
            for kc in range(ECHUNK):
                nc.tensor.matmul(
                    ps[:, 0:NH * D],
                    xv_sb[kc][:, 128 * mk:128 * (mk + 1)],
                    wv_sb[kc][:],
                    start=(kc == 0),
                    stop=(kc == ECHUNK - 1),
                )
            t = vsb[mk]
            vsrc = ps[:, 0:NH * D].rearrange("p (h d) -> p h d", h=NH)
            vdst = t[:].rearrange("p (h d) -> p h d", h=NH)[:, :, 0:D]
            nc.vector.tensor_copy(vdst, vsrc)
            ones_cols = t[:].rearrange("p (h d) -> p h d", h=NH)[:, :, D:D + 1]
            nc.vector.memset(ones_cols, 1.0)

        # ---- attention state ----
        oall = [oallp.tile([128, NH * D], DT, tag=f"oall{m}", name="oall") for m in range(NCHUNK)]
        ot = [otp.tile([128, N], DT, tag=f"ot{c}", name="ot") for c in range(2)]

        acc_tiles = {}  # head -> [3 tiles]

        def acc_ap(i, m):
            t = acc_tiles[i][m // 6]
            s = ACC_STRIDE * (m % 6)
            return t[:, s:s + 65]

        def emit_energy_half(i, kc, qh, att):
            """One query half of attT[kc] for head i on its own PE row group
            (0:63 for qh=0, 64:127 for qh=1 via the dup copies)."""
            ps = ps_big.tile([128, 1024], F32, tag="big", name="pse")
            kh = half_ap(knd, kdp, i, 64 * qh)
            qa = half_ap(qnd, qdp, i, 64 * qh)
            for j in range(2):
                nc.tensor.matmul(
                    ps[:, 512 * j:512 * (j + 1)],
                    kh[:, 128 * kc:128 * (kc + 1)],
                    qa[:, 1024 * qh + 512 * j:1024 * qh + 512 * (j + 1)],
                    start=True,
                    stop=True,
                )
            nc.scalar.activation(
                att[:, 1024 * qh:1024 * (qh + 1)], ps[:], AF.Exp, scale=SCALE
            )

        def emit_energy_kc_half(i, kc, qh, att):
            """One query half of attT[kc] on PE rows 0:63 only (natural
            layout, no dup/swap dependency) — used for head 0's two sweeps."""
            ps = ps_big.tile([128, 1024], F32, tag="big", name="pse")
            kh = half_ap(knd, kdp, i, 0)
            qa = half_ap(qnd, qdp, i, 0)
            for j in range(2):
                nc.tensor.matmul(
                    ps[:, 512 * j:512 * (j + 1)],
                    kh[:, 128 * kc:128 * (kc + 1)],
                    qa[:, 1024 * qh + 512 * j:1024 * qh + 512 * (j + 1)],
                    start=True,
                    stop=True,
                )
            nc.scalar.activation(
                att[:, 1024 * qh:1024 * (qh + 1)], ps[:], AF.Exp, scale=SCALE
            )

        def emit_attv_half(i, kc, att_tile, mhalf):
            """Half a kc step of head i's att@v (m-chunks mhalf*8..mhalf*8+7):
            accumulate into the packed accumulators.  start=True only on the
            very first matmul touching each PSUM bank (pending-zero covers
            the other chains); stop=True on the last matmul into the bank."""
            if kc == 0 and mhalf == 0:
                acc_tiles[i] = [
                    ps_acc.tile([128, 512], F32, tag=f"acc{j}", name="acc")
                    for j in range(3)
                ]
            for m in range(8 * mhalf, 8 * mhalf + 8):
                first_in_bank = kc == 0 and (m % 6) == 0
                last_in_bank = kc == NCHUNK - 1 and (m % 6 == 5 or m == NCHUNK - 1)
                nc.tensor.matmul(
                    acc_ap(i, m),
                    att_tile[:, 128 * m:128 * (m + 1)],
                    vsb[kc][:, 65 * i:65 * i + 65],
                    start=first_in_bank,
                    stop=last_in_bank,
                )

        def emit_attv_step(i, kc, att_tile):
            emit_attv_half(i, kc, att_tile, 0)
            emit_attv_half(i, kc, att_tile, 1)

        def emit_normalize(i):
            # batched reciprocals: one strided gather per accumulator bank
            # (denominator columns sit at 64 + 80*j), then 16 per-partition
            # scalar multiplies.
            rec = smallp.tile([128, NCHUNK], F32, tag="rec", name="rec")
            for j, lo, cnt in ((0, 0, 6), (1, 6, 6), (2, 12, 4)):
                t = acc_tiles[i][j]
                dsrc = t[:, 0:ACC_STRIDE * cnt].rearrange(
                    "p (a b) -> p a b", b=ACC_STRIDE
                )[:, :, 64:65]
                ddst = rec[:, lo:lo + cnt].rearrange("p (a b) -> p a b", b=1)
                nc.vector.reciprocal(ddst, dsrc)
            for m in range(NCHUNK):
                pav = acc_ap(i, m)
                nc.vector.tensor_scalar_mul(
                    oall[m][:, D * i:D * (i + 1)], pav[:, 0:D], rec[:, m:m + 1]
                )

        def emit_warm(n_mm=10):
            """Dense FD=512 matmuls with a reused stationary operand: a
            continuous PE streaming burst that flips the HAM clock gate back
            to 2.4 GHz after a slot-boundary stall."""
            ps = ps_big.tile([128, 1024], F32, tag="big", name="warm")
            for t in range(n_mm):
                nc.tensor.matmul(
                    ps[:, 512 * (t % 2):512 * (t % 2 + 1)],
                    iden_sb[:],
                    wo_sb[0][:],
                    start=True,
                    stop=True,
                )

        def emit_tail(m):
            """PE-transpose oall[m] into ot and apply the Wo slice.  Uses a
            ps_big tile: [0:512] Wo accum, [512:640]/[640:768] transposes.
            The PSUM->SBUF copies go to the (now idle) scalar engine so the
            vector engine only carries the normalize + output stage."""
            pb = ps_big.tile([128, 1024], F32, tag="big", name="pbt")
            for c in range(2):
                # fp16 view of 64 f32 lanes in the second bank
                pt = pb[:, 512 + 64 * c:512 + 64 * (c + 1)].bitcast(DT)
                nc.tensor.transpose(pt, oall[m][:, 128 * c:128 * (c + 1)], iden_sb[:])
                nc.scalar.copy(ot[c][:, 128 * m:128 * (m + 1)], pt)
            pf = pb[:, 0:512]
            for c in range(2):
                nc.tensor.matmul(
                    pf,
                    ot[c][:, 128 * m:128 * (m + 1)],
                    wo_sb[c][:],
                    start=(c == 0),
                    stop=(c == 1),
                )
            st = ostage.tile([128, E], F32, tag="st", name="st")
            nc.vector.tensor_copy(st[:], pf)
            nc.gpsimd.dma_start(out=out[128 * m:128 * (m + 1), :], in_=st[:])

        # ================= schedule =================
        # slot 0 (head 0): minimal critical path — q-proj(mc0, half 0) +
        # k-proj(mc0, block 0) gate the first exp of the Q0 sweep; everything
        # else (q half 1, mc1 projections, swaps, v-proj, remaining k blocks)
        # is woven in behind it.  Head 0's energy runs serially on PE rows
        # 0:63 so it needs no dup-swap at all.
        emit_qproj_half(0, 0)
        emit_kproj_block(0, 0)
        att_tiles = {}
        for kc in range(NCHUNK):
            att_tiles[(0, kc)] = big.tile([128, N], DT, tag="big", name="att")

        for kc in range(NCHUNK):
            if kc in (4, 8, 12):
                emit_kproj_block(kc // 4, 0)
            emit_energy_kc_half(0, kc, 0, att_tiles[(0, kc)])
            if kc == 0:
                emit_qproj_half(0, 1)

        # Q1 sweep, with the rest of the projection work woven in
        for kc in range(NCHUNK):
            emit_energy_kc_half(0, kc, 1, att_tiles[(0, kc)])
            if kc == 0:
                emit_qswap(0)
                emit_qproj_half(1, 0)
            elif kc == 1:
                emit_qproj_half(1, 1)
            elif kc == 2:
                emit_qswap(1)
            elif kc in (3, 4, 5, 6):
                emit_kproj_block(kc - 3, 1)
            elif kc >= 7 and kc <= 14:
                emit_vproj_chunk(2 * (kc - 7))
                emit_vproj_chunk(2 * (kc - 7) + 1)

        # slots 1-2: energy+exp(head i) with att@v(head i-1) interleaved at
        # lag 2 so the i-1 normalize overlaps slot boundaries.  Emission per
        # kc is [E(A) | attv half | E(B) | attv half] so the PE always has
        # ~2us of work between an exp and the energy matmul that reuses its
        # PSUM slot (removes the per-kc PE stall that kept HAM cold).
        for i in (1, 2):
            for kc in range(NCHUNK):
                att = att_tiles[(i, kc)] = big.tile([128, N], DT, tag="big", name="att")
                emit_energy_half(i, kc, 0, att)
                if kc >= 2:
                    emit_attv_half(i - 1, kc - 2, att_tiles[(i - 1, kc - 2)], 0)
                emit_energy_half(i, kc, 1, att)
                if kc >= 2:
                    emit_attv_half(i - 1, kc - 2, att_tiles.pop((i - 1, kc - 2)), 1)
            for kc in (NCHUNK - 2, NCHUNK - 1):
                emit_attv_step(i - 1, kc, att_tiles.pop((i - 1, kc)))
            emit_normalize(i - 1)
            emit_warm()

        # slot 3: attv(h2) front-loaded (its att tiles all exist), then
        # normalize(h2), then attv(h3) woven in at catch-up pace so only one
        # step + normalize + tail remain after the last exp.
        for kc in range(NCHUNK):
            att = att_tiles[(3, kc)] = big.tile([128, N], DT, tag="big", name="att")
            emit_energy_half(3, kc, 0, att)
            if kc < 6:
                emit_attv_step(2, 2 * kc, att_tiles.pop((2, 2 * kc)))
            elif kc == 6:
                emit_attv_step(2, 12, att_tiles.pop((2, 12)))
                emit_attv_step(2, 14, att_tiles.pop((2, 14)))
            emit_energy_half(3, kc, 1, att)
            if kc < 6:
                emit_attv_step(2, 2 * kc + 1, att_tiles.pop((2, 2 * kc + 1)))
            elif kc == 6:
                emit_attv_step(2, 13, att_tiles.pop((2, 13)))
                emit_attv_step(2, 15, att_tiles.pop((2, 15)))
            elif kc == 7:
                emit_normalize(2)
                emit_warm()
            else:
                emit_attv_step(3, 2 * (kc - 8), att_tiles.pop((3, 2 * (kc - 8))))
                emit_attv_step(3, 2 * (kc - 8) + 1, att_tiles.pop((3, 2 * (kc - 8) + 1)))

        emit_normalize(3)
        for m in range(NCHUNK):
            emit_tail(m)

    nc.compile()
    _built = nc
    return nc


def _host_prep(query, key, value, Wq, Wk, Wv, Wo, c):
    b, g = c // 2, c % 2
    DT = np.float16
    wqtd = np.empty((E, 256), np.float32)
    wktd = np.empty((E, 256), np.float32)
    wvt = np.empty((E, NH * D), np.float32)
    wot = np.empty((NH * D, E), np.float32)
    for i in range(NH):
        h = NH * g + i
        wqtd[:, D * i:D * (i + 1)] = Wq[D * h:D * (h + 1), :].T
        wktd[:, D * i:D * (i + 1)] = Wk[D * h:D * (h + 1), :].T
        wvt[:, D * i:D * (i + 1)] = Wv[D * h:D * (h + 1), :].T
        wot[D * i:D * (i + 1), :] = Wo[:, D * h:D * (h + 1)].T
    return {
        "xqT": np.ascontiguousarray(query[b].T).astype(DT),
        "xkT": np.ascontiguousarray(key[b].T).astype(DT),
        "xvT": np.ascontiguousarray(value[b].T).astype(DT),
        "wqtd": wqtd.astype(DT),
        "wktd": wktd.astype(DT),
        "wvt": wvt.astype(DT),
        "wot": wot.astype(DT),
        "iden": np.eye(128, dtype=DT),
    }


# test.py can flip these to profile
TRACE = False
TRACE_KWARGS = {}
LAST_RESULTS = None


def kernel(query, key, value, Wq, Wk, Wv, Wo):
    global LAST_RESULTS
    from concourse.bass_utils import run_bass_kernel_spmd

    args = [np.asarray(x, dtype=np.float32) for x in (query, key, value, Wq, Wk, Wv, Wo)]
    nc = _build()
    in_maps = [_host_prep(*args, c) for c in range(N_CORES)]
    res = run_bass_kernel_spmd(
        nc, in_maps, core_ids=list(range(N_CORES)), trace=TRACE, **TRACE_KWARGS
    )
    LAST_RESULTS = res
    outp = np.zeros((B, N, E), np.float32)
    for c in range(N_CORES):
        outp[c // 2] += res.results[c]["out"]
    return outp


# revision 22
# speedup vs baseline: 1.2058x; 1.2058x over previous
"""Multi-head attention (B=4, N=2048, E=512, H=8) on 8 Trainium2 cores.

Sharding: core c -> (batch b = c//2, head-group g = c%2 of 4 heads).
Each core computes q/k/v projections for its 4 heads, full attention,
and a partial output projection (its heads' slice of Wo contraction);
the host sums the two partials per batch.

Device data flow (per core, all matmul inputs fp16, accumulation f32):
  - host supplies transposed inputs xqT/xkT/xvT [E, N] and weight slices
    (q/k weights dup-interleaved so each head's projection lands as a
    [128, N] tile with the head's 64 dims duplicated in both partition
    halves -> row-tiled (64-row) energy matmuls run pairwise-concurrent
    at full PE rate)
  - energy (transposed): attT[kc] [128(nk), 2048(nq)] = exp(k_chunk @ qT * s)
    via single K=64 matmuls, exp on ACT straight from PSUM (FD=2048)
  - att @ v_aug: v has a ones column appended, so one accumulated matmul
    chain yields [nq, 64] unnormalized output AND the softmax denominator
  - normalize with per-partition reciprocal (DVE), PE-transpose the
    [nq, 256] result, output projection against WoT slice.
"""

import sys

if "/opt/trn_rl_repo" not in sys.path:
    sys.path.insert(0, "/opt/trn_rl_repo")

import numpy as np

B, N, E, H, D = 4, 2048, 512, 8, 64
NH = 4                      # heads per core
NCHUNK = N // 128           # 16
ECHUNK = E // 128           # 4
SCALE = float(1.0 / np.sqrt(E))
N_CORES = 8

ATT_POOL_BUFS = 34          # shared [128,2048] fp16 slots: 12 xT tiles + 2-head attT window

_built = None


def _build():
    global _built
    if _built is not None:
        return _built

    from contextlib import ExitStack

    import concourse.bass as bass  # noqa: F401
    import concourse.mybir as mybir
    import concourse.tile as tile
    from concourse import bacc

    DT = mybir.dt.float16
    F32 = mybir.dt.float32
    AF = mybir.ActivationFunctionType

    nc = bacc.Bacc(
        "TRN2",
        target_bir_lowering=False,
        debug=False,
        num_devices=N_CORES,
    )

    xqT = nc.dram_tensor("xqT", [E, N], DT, kind="ExternalInput").ap()
    xkT = nc.dram_tensor("xkT", [E, N], DT, kind="ExternalInput").ap()
    xvT = nc.dram_tensor("xvT", [E, N], DT, kind="ExternalInput").ap()
    wqtd = nc.dram_tensor("wqtd", [E, 256], DT, kind="ExternalInput").ap()
    wktd = nc.dram_tensor("wktd", [E, 256], DT, kind="ExternalInput").ap()
    wvt = nc.dram_tensor("wvt", [E, NH * D], DT, kind="ExternalInput").ap()
    wot = nc.dram_tensor("wot", [NH * D, E], DT, kind="ExternalInput").ap()
    iden = nc.dram_tensor("iden", [128, 128], DT, kind="ExternalInput").ap()
    out = nc.dram_tensor("out", [N, E], F32, kind="ExternalOutput").ap()

    with tile.TileContext(nc) as tc, ExitStack() as ctx:
        consts = ctx.enter_context(tc.tile_pool(name="consts", bufs=1))
        big = ctx.enter_context(tc.tile_pool(name="big", bufs=ATT_POOL_BUFS))
        qk = ctx.enter_context(tc.tile_pool(name="qk", bufs=1))
        vp = ctx.enter_context(tc.tile_pool(name="vp", bufs=1))
        oallp = ctx.enter_context(tc.tile_pool(name="oall", bufs=1))
        otp = ctx.enter_context(tc.tile_pool(name="ot", bufs=1))
        ostage = ctx.enter_context(tc.tile_pool(name="ostage", bufs=3))
        smallp = ctx.enter_context(tc.tile_pool(name="small", bufs=4))

        # PSUM budget (8 banks): energy fp16 [128,2048] = 2 banks x2 bufs,
        # attv [128,65] = 1 bank x2, fin/proj/transpose [128,512]f32 = 1 bank x2
        ps_big = ctx.enter_context(tc.tile_pool(name="ps_big", bufs=3, space="PSUM"))
        ps_av = ctx.enter_context(tc.tile_pool(name="ps_av", bufs=2, space="PSUM"))
        ps_fin = ps_av  # share the same 2 banks (tag-distinct tiles)

        # ---- constant / weight loads ----
        iden_sb = consts.tile([128, 128], DT, tag="iden", name="iden_sb")
        nc.sync.dma_start(out=iden_sb[:], in_=iden[:])
        wq_sb = [consts.tile([128, 256], DT, tag=f"wq{kc}", name=f"wq_sb{kc}") for kc in range(ECHUNK)]
        wk_sb = [consts.tile([128, 256], DT, tag=f"wk{kc}", name=f"wk_sb{kc}") for kc in range(ECHUNK)]
        wv_sb = [consts.tile([128, NH * D], DT, tag=f"wv{kc}", name=f"wv_sb{kc}") for kc in range(ECHUNK)]
        wo_sb = [consts.tile([128, E], DT, tag=f"wo{c}", name=f"wo_sb{c}") for c in range(2)]
        for kc in range(ECHUNK):
            nc.sync.dma_start(out=wq_sb[kc][:], in_=wqtd[128 * kc:128 * (kc + 1), :])
            nc.sync.dma_start(out=wk_sb[kc][:], in_=wktd[128 * kc:128 * (kc + 1), :])
            nc.sync.dma_start(out=wv_sb[kc][:], in_=wvt[128 * kc:128 * (kc + 1), :])
        for c in range(2):
            nc.sync.dma_start(out=wo_sb[c][:], in_=wot[128 * c:128 * (c + 1), :])

        # ---- activation inputs (transposed on host) ----
        # chain the three tensors' loads so xq gets full HBM bandwidth first
        # (q-proj starts ~8us earlier), then xk, then xv
        from concourse.tile_rust import add_dep_helper

        xq_sb, xk_sb, xv_sb = [], [], []
        last_dma = None
        for (src_ap, outl) in ((xqT, xq_sb), (xkT, xk_sb), (xvT, xv_sb)):
            first = None
            for kc in range(ECHUNK):
                t = big.tile([128, N], DT, tag="big", name="xin")
                inst = nc.sync.dma_start(out=t[:], in_=src_ap[128 * kc:128 * (kc + 1), :])
                if first is None:
                    first = inst
                outl.append(t)
            last_dma = inst

        # ---- q/k projections ----
        # qnd[mc]/knd[mc] [128, N]: natural head-pair chunks (head 2mc at
        # rows 0:64, head 2mc+1 at 64:128). qdp/kdp are the swapped copies
        # (made by 2 sbuf->sbuf DMAs) so each head has its 64 dims available
        # in BOTH partition halves -> row-tiled energy matmuls at full rate.
        qnd = [qk.tile([128, N], DT, tag=f"qnd{mc}", name="qnd") for mc in range(2)]
        knd = [qk.tile([128, N], DT, tag=f"knd{mc}", name="knd") for mc in range(2)]
        qdp = [qk.tile([128, N], DT, tag=f"qdp{mc}", name="qdp") for mc in range(2)]
        kdp = [qk.tile([128, N], DT, tag=f"kdp{mc}", name="kdp") for mc in range(2)]

        def emit_proj_qk(mc):
            for (w_sb, x_sb, nd, dp) in (
                (wq_sb, xq_sb, qnd, qdp),
                (wk_sb, xk_sb, knd, kdp),
            ):
                for ns in range(4):
                    ps = ps_fin.tile([128, 512], F32, tag="av", name="ps")
                    for kc in range(ECHUNK):
                        nc.tensor.matmul(
                            ps[:],
                            w_sb[kc][:, 128 * mc:128 * (mc + 1)],
                            x_sb[kc][:, 512 * ns:512 * (ns + 1)],
                            start=(kc == 0),
                            stop=(kc == ECHUNK - 1),
                        )
                    nc.vector.tensor_copy(nd[mc][:, 512 * ns:512 * (ns + 1)], ps[:])
                nc.gpsimd.dma_start(out=dp[mc][0:64, :], in_=nd[mc][64:128, :])
                nc.gpsimd.dma_start(out=dp[mc][64:128, :], in_=nd[mc][0:64, :])

        def half_ap(nd, dp, i, half):
            """[64, N] view of head i's projected data at partition `half`."""
            mc, r = divmod(i, 2)
            if half == 0:
                t = nd[mc] if r == 0 else dp[mc]
                return t[0:64, :]
            t = dp[mc] if r == 0 else nd[mc]
            return t[64:128, :]

        # ---- v projection into augmented layout vsb[kc] [128, NH*65] ----
        # col 65*i + 64 is the ones column for head i (softmax denominator).
        vsb = []

        def emit_vproj():
            for mk in range(NCHUNK):
                ps = ps_fin.tile([128, E], F32, tag="av", name="psf")
                for kc in range(ECHUNK):
                    nc.tensor.matmul(
                        ps[:, 0:NH * D],
                        xv_sb[kc][:, 128 * mk:128 * (mk + 1)],
                        wv_sb[kc][:],
                        start=(kc == 0),
                        stop=(kc == ECHUNK - 1),
                    )
                t = vp.tile([128, NH * 65], DT, tag=f"v{mk}", name=f"v_sb{mk}")
                vsrc = ps[:, 0:NH * D].rearrange("p (h d) -> p h d", h=NH)
                vdst = t[:].rearrange("p (h d) -> p h d", h=NH)[:, :, 0:D]
                nc.vector.tensor_copy(vdst, vsrc)
                ones_cols = t[:].rearrange("p (h d) -> p h d", h=NH)[:, :, D:D + 1]
                nc.vector.memset(ones_cols, 1.0)
                vsb.append(t)

        # ---- attention ----
        oall = [oallp.tile([128, NH * D], DT, tag=f"oall{m}", name=f"oall{m}") for m in range(NCHUNK)]
        ot = [otp.tile([128, N], DT, tag=f"ot{c}", name=f"ot{c}") for c in range(2)]

        def emit_energy(i):
            """attT tiles for head i: exp(k_chunk @ q_h.T * SCALE), [128, nq]."""
            tiles = []
            for kc in range(NCHUNK):
                # two [128,1024] f32 psum tiles (2 banks each) per chunk, one
                # per PE row group (partition halves run as concurrent
                # row-tiled matmuls); bufs=2 keeps ACT streaming while PE
                # fills the next chunk
                att = big.tile([128, N], DT, tag="big", name="att")
                for half, ns in ((0, 0), (64, 1)):
                    ps = ps_big.tile([128, N // 2], F32, tag="big", name="ps")
                    kh = half_ap(knd, kdp, i, half)
                    qh = half_ap(qnd, qdp, i, half)
                    for j in range(2):
                        nc.tensor.matmul(
                            ps[:, 512 * j:512 * (j + 1)],
                            kh[:, 128 * kc:128 * (kc + 1)],
                            qh[:, 1024 * ns + 512 * j:1024 * ns + 512 * (j + 1)],
                            start=True,
                            stop=True,
                        )
                    nc.scalar.activation(
                        att[:, 1024 * ns:1024 * (ns + 1)], ps[:], AF.Exp, scale=SCALE
                    )
                tiles.append(att)
            return tiles

        def emit_tail(m):
            """PE-transpose oall[m] into ot and apply the Wo slice."""
            for c in range(2):
                pt = ps_fin.tile([128, 128], DT, tag="av", name="pt")
                nc.tensor.transpose(pt[:], oall[m][:, 128 * c:128 * (c + 1)], iden_sb[:])
                nc.scalar.copy(ot[c][:, 128 * m:128 * (m + 1)], pt[:])
            pf = ps_fin.tile([128, E], F32, tag="av", name="pff")
            for c in range(2):
                nc.tensor.matmul(
                    pf[:],
                    ot[c][:, 128 * m:128 * (m + 1)],
                    wo_sb[c][:],
                    start=(c == 0),
                    stop=(c == 1),
                )
            st = ostage.tile([128, E], F32, tag="st", name="st")
            nc.vector.tensor_copy(st[:], pf[:])
            nc.sync.dma_start(out=out[128 * m:128 * (m + 1), :], in_=st[:])

        def emit_attv(i, att_tiles, fuse_tail=False):
            for m in range(NCHUNK):
                pav = ps_av.tile([128, 65], F32, tag="av", name="pav")
                for kc in range(NCHUNK):
                    nc.tensor.matmul(
                        pav[:],
                        att_tiles[kc][:, 128 * m:128 * (m + 1)],
                        vsb[kc][:, 65 * i:65 * i + 65],
                        start=(kc == 0),
                        stop=(kc == NCHUNK - 1),
                    )
                rec = smallp.tile([128, 1], F32, tag="rec", name="rec")
                nc.vector.reciprocal(rec[:], pav[:, 64:65])
                nc.vector.tensor_scalar_mul(
                    oall[m][:, D * i:D * (i + 1)], pav[:, 0:D], rec[:]
                )
                if fuse_tail:
                    emit_tail(m)

        def emit_warm(n_mm=12):
            """Dense N=512 matmuls with a reused stationary operand: ~100%
            PE-busy streak that flips the HAM clock gate to 2.4 GHz."""
            ps = ps_fin.tile([128, 512], F32, tag="av", name="warm")
            for _ in range(n_mm):
                nc.tensor.matmul(ps[:], iden_sb[:], wo_sb[0][:], start=True, stop=True)

        # software-pipelined emission: head i's energy feeds ACT while PE
        # fills gaps with projections and head i-1's att@v
        emit_proj_qk(0)
        att0 = emit_energy(0)
        emit_proj_qk(1)
        emit_vproj()
        emit_warm()
        att1 = emit_energy(1)
        emit_attv(0, att0)
        emit_warm()
        att2 = emit_energy(2)
        emit_attv(1, att1)
        emit_warm()
        att3 = emit_energy(3)
        emit_attv(2, att2)
        emit_warm()
        emit_attv(3, att3, fuse_tail=True)

    nc.compile()
    _built = nc
    return nc


def _host_prep(query, key, value, Wq, Wk, Wv, Wo, c):
    b, g = c // 2, c % 2
    DT = np.float16
    wqtd = np.empty((E, 256), np.float32)
    wktd = np.empty((E, 256), np.float32)
    wvt = np.empty((E, NH * D), np.float32)
    wot = np.empty((NH * D, E), np.float32)
    for i in range(NH):
        h = NH * g + i
        wqtd[:, D * i:D * (i + 1)] = Wq[D * h:D * (h + 1), :].T
        wktd[:, D * i:D * (i + 1)] = Wk[D * h:D * (h + 1), :].T
        wvt[:, D * i:D * (i + 1)] = Wv[D * h:D * (h + 1), :].T
        wot[D * i:D * (i + 1), :] = Wo[:, D * h:D * (h + 1)].T
    return {
        "xqT": np.ascontiguousarray(query[b].T).astype(DT),
        "xkT": np.ascontiguousarray(key[b].T).astype(DT),
        "xvT": np.ascontiguousarray(value[b].T).astype(DT),
        "wqtd": wqtd.astype(DT),
        "wktd": wktd.astype(DT),
        "wvt": wvt.astype(DT),
        "wot": wot.astype(DT),
        "iden": np.eye(128, dtype=DT),
    }


# test.py can flip these to profile
TRACE = False
TRACE_KWARGS = {}
LAST_RESULTS = None


def kernel(query, key, value, Wq, Wk, Wv, Wo):
    global LAST_RESULTS
    from concourse.bass_utils import run_bass_kernel_spmd

    args = [np.asarray(x, dtype=np.float32) for x in (query, key, value, Wq, Wk, Wv, Wo)]
    nc = _build()
    in_maps = [_host_prep(*args, c) for c in range(N_CORES)]
    res = run_bass_kernel_spmd(
        nc, in_maps, core_ids=list(range(N_CORES)), trace=TRACE, **TRACE_KWARGS
    )
    LAST_RESULTS = res
    outp = np.zeros((B, N, E), np.float32)
    for c in range(N_CORES):
        outp[c // 2] += res.results[c]["out"]
    return outp
